# revision 22
# baseline (speedup 1.0000x reference)
"""Trainium2 Bass kernel for nn_ErrorAwareEdgeLoss.

reference:  cost[b,e] = sum_{p,q} P[b,i_e,p] * d_error[p,q] * P[b,j_e,q]
            result    = mean_{b,e} cost[b,e]

The edge pairs only enter through the count matrix
    C[l1,l2] = #edges e with (i_e,j_e) == (l1,l2),
and since d_error is symmetric the result collapses to
    result = <d_error, sum_b Q_b^T Cs Q_b> / (B*E),
with Q_b = P[b,:64,:] and Cs = (C + C^T)/2 (exact in fp8: half-integers).

Device work per core (256 batches, data-parallel over batch), all in fp8
(e4m3, host-packed as 64*Q — the softmax values are tiny so 64*Q stays
well inside [2^-6, 240] and the final scale divides by 64^2; empirical
rel err vs f64 reference ~2e-3, tolerance 2e-2):
  - ONE host-packed data stream `qw`, laid out in DoubleRowSwInterleave
    weight order (per 2-pair block kb, byte t = 2j+i holds
    64*Q[pair 2kb+i][p, 127-j]); block 0 carries blockdiag(Cs,Cs):
      * Y matmuls stream it as the moving operand (columns come out in
        (block, 127-h, pair-half) order, which just permutes PSUM
        columns),
      * R matmuls load it as SwInterleave weights — the contiguous
        weight read paces an R matmul at ~58ns vs ~77ns for the HW
        DoubleRow interleave, with no extra HBM traffic,
      * the host flips d_error's columns to absorb the reversed h
        order of R's output columns.
  - warmup matmuls on a zeroed scratch tile start the PE before any DMA
    lands (the HAM clock gate needs ~3.4us of sustained PE activity to
    lift the PE from 1.2 to 2.4GHz); scratch is memset on GPSIMD, which
    is the first engine free after the tile-context preamble
  - group loads all go on the ONE sync HWDGE ring so they complete
    strictly in consumption order; the lead group also carries the cs
    block so one descriptor batch covers both
  - Y = Cs @ Q via ONE blockdiag(Cs,Cs) matmul per 512-wide slab;
    PSUM->SBUF fp8 casts alternate between DVE and ACT as whole slabs
  - R += Q^T Y with K=256 fp8 DoubleRowSwInterleave matmuls (two
    batch-pairs per instruction) accumulated in PSUM f32 — R-matmuls
    run one group behind the Y-matmuls so the PE never waits on the
    casts
  - final reduce ON DEVICE: s[p] = sum_q R[p,q]*d_flip[p,q] via one DVE
    scalar_tensor_tensor (PSUM x SBUF) with a bf16 accumulator, then a
    1-column bf16 matmul against ones folds partitions, and a single
    4-byte result is written out
Host: result = sum_c scalar_c / (B*E*64^2).
"""

import sys

_TRN_REPO = "/opt/trn_rl_repo"
if _TRN_REPO not in sys.path:
    sys.path.insert(0, _TRN_REPO)

import numpy as np
import ml_dtypes

B, L, H = 2048, 64, 128     # batch, logical qubits, physical dim
E = 512                     # number of circuit edges
N_CORES = 8
BPC = B // N_CORES          # 256 batches per core
NPAIRS = BPC // 2           # 128 batch-pairs per core
NB = NPAIRS // 2            # 64 two-pair blocks per core
GROUPS = [8, 8] + [16] * 6 + [8, 8]  # pairs per load: small lead-in groups
                            # start the PE sooner, small tail groups cut
                            # the trailing R run; sum must be NPAIRS
SLAB_PAIRS = 4              # pairs per Y-matmul slab (512 moving columns)
QSCALE = 64.0               # host-side scale before fp8 cast
WARMUP_MM = 22              # dummy matmuls that keep the PE busy (ramping
                            # the HAM clock gate) until the first group
                            # load lands
D_LOAD_AFTER = 7            # emission point of the d_error load on the
                            # GPSIMD software-DGE queue (idle engine, so
                            # it never delays the sync ring)

_CACHE = {}


def _build():
    import concourse.tile as tile
    from concourse import bacc, mybir

    f32 = mybir.dt.float32
    fp8 = mybir.dt.float8e4

    nc = bacc.Bacc(None)
    # Host-packed stream: qw[p, 0, 0, :] = blockdiag(Cs,Cs)[p, :] and for
    # kb >= 1, flat byte t = 2j+i of block kb is
    # 64*Q[2*(kb-1) + i -th pair of partition p's half][p%64, 127-j].
    qw = nc.dram_tensor("qw", [128, 1 + NB, 2, H], fp8, kind="ExternalInput")
    dmat = nc.dram_tensor("dmat", [H, H], f32, kind="ExternalInput")
    res_out = nc.dram_tensor("res_out", [128, 1], mybir.dt.bfloat16,
                             kind="ExternalOutput")

    with tile.TileContext(nc) as tc:
        with (
            tc.tile_pool(name="singles", bufs=1) as singles,
            tc.tile_pool(name="qbfp", bufs=10) as qbf_pool,
            tc.tile_pool(name="ybfp", bufs=3) as ybf_pool,
            tc.tile_pool(name="yps", bufs=6, space="PSUM") as yps,
            tc.tile_pool(name="rps", bufs=1, space="PSUM") as rps,
            tc.tile_pool(name="wps", bufs=1, space="PSUM") as wps,
        ):
            # PE p-state warmup: matmuls on a zeroed scratch tile, no DMA
            # dependency, so the HAM ramp starts as early as possible.
            scratch = singles.tile([128, 128], fp8)
            nc.gpsimd.memset(scratch[:, :], 0)
            bf16 = mybir.dt.bfloat16
            wm_psum = wps.tile([128, 128], f32)
            for _ in range(WARMUP_MM):
                nc.tensor.matmul(
                    wm_psum[:, :], lhsT=scratch[:, :], rhs=scratch[:, :],
                    start=True, stop=True, skip_group_check=True,
                )

            d_sb = singles.tile([H, H], f32)
            r_psum = rps.tile([128, H], f32)

            # All group loads go on the ONE sync-queue ring: SDMA engines
            # drain a ring FIFO, so groups complete in consumption order.
            # Group 0 also carries the cs block (kb=0).
            def load_group(b0, nblk, lead):
                if lead:
                    qbf = qbf_pool.tile([128, 1 + nblk, 2, H], fp8)
                    nc.sync.dma_start(
                        out=qbf[:, :, :, :], in_=qw[:, 0 : 1 + nblk, :, :]
                    )
                    return qbf
                qbf = qbf_pool.tile([128, nblk, 2, H], fp8)
                nc.sync.dma_start(
                    out=qbf[:, :, :, :], in_=qw[:, 1 + b0 : 1 + b0 + nblk, :, :]
                )
                return qbf

            _state = {"first": True, "cs2": None}

            def emit_y_slab(qbf, ybf, s, boff):
                yy = yps.tile([128, SLAB_PAIRS * H], f32)
                b = boff + s * (SLAB_PAIRS // 2)
                nc.tensor.matmul(
                    yy[:, :], lhsT=_state["cs2"],
                    rhs=qbf[:, b : b + SLAB_PAIRS // 2, :, :],
                    start=True, stop=True, skip_group_check=True,
                )
                # PSUM -> SBUF fp8 cast: whole slab on ONE engine,
                # alternating DVE/ACT per slab; a plain linear copy, so
                # ybf inherits the interleaved column order.
                eng = nc.vector.tensor_copy if s % 2 == 0 else nc.scalar.copy
                sb = s * (SLAB_PAIRS // 2)
                eng(ybf[:, sb : sb + SLAB_PAIRS // 2, :, :], yy[:, :])

            def emit_r_block(qwbuf, ybf, kb, boff, last):
                from concourse import mybir as mb

                first = _state["first"]
                _state["first"] = False
                nc.tensor.matmul(
                    r_psum[:, :],
                    lhsT=qwbuf[:, boff + kb, :, :],
                    rhs=ybf[:, kb, :, :].transpose([0, 2, 1]),
                    start=first, stop=last, skip_group_check=True,
                    perf_mode=mb.MatmulPerfMode.DoubleRowSwInterleave,
                )

            # Software pipeline: R-matmuls run one group behind the
            # Y-matmuls so the PE never waits on the PSUM->SBUF casts.
            assert sum(GROUPS) == NPAIRS
            prev = None
            b0 = 0
            for gi, npairs in enumerate(GROUPS):
                nblk = npairs // 2
                qbf = load_group(b0, nblk, lead=(gi == 0))
                boff = 1 if gi == 0 else 0
                if gi == 0:
                    _state["cs2"] = qbf[:, 0, 0, :]
                b0 += nblk
                # ybf keeps the Y-stream byte order: [block, j, i]
                ybf = ybf_pool.tile([128, nblk, H, 2], fp8)
                for s in range(npairs // SLAB_PAIRS):
                    emit_y_slab(qbf, ybf, s, boff)
                if prev is not None:
                    qw_, py_, po_, pn_ = prev
                    for kb in range(pn_):
                        emit_r_block(qw_, py_, kb, po_, last=False)
                if gi == D_LOAD_AFTER:
                    # software DGE on the idle GPSIMD engine: keeps the
                    # 64KB d_error load entirely off the sync ring that
                    # feeds the PE
                    nc.gpsimd.dma_start(out=d_sb[:, :], in_=dmat[:, :])
                prev = (qbf, ybf, boff, nblk)
            qw_, py_, po_, pn_ = prev
            for kb in range(pn_):
                emit_r_block(qw_, py_, kb, po_, last=(kb == pn_ - 1))

            # On-device <d_flip, R>: per-partition dot on DVE (bf16
            # accumulator), written out directly — the host sums the
            # 128 partials, which is cheaper than a fold-matmul +
            # PSUM copy on the critical tail path.
            prod = singles.tile([128, H], f32)
            acc16 = singles.tile([128, 1], bf16)
            nc.vector.scalar_tensor_tensor(
                out=prod[:, :], in0=r_psum[:, :], scalar=1.0, in1=d_sb[:, :],
                op0=mybir.AluOpType.mult, op1=mybir.AluOpType.mult,
                accum_out=acc16[:, :],
            )
            nc.sync.dma_start(out=res_out[:, :], in_=acc16[:, :])

    nc.compile()
    return nc


def get_nc():
    key = ("nc", "fp8v6")
    if key not in _CACHE:
        _CACHE[key] = _build()
    return _CACHE[key]


def make_count_matrix(circuit_edge_pairs):
    pairs = np.asarray(circuit_edge_pairs).astype(np.int64)
    C = np.zeros((L, L), np.float64)
    np.add.at(C, (pairs[:, 0], pairs[:, 1]), 1.0)
    Cs = (C + C.T) * 0.5
    cs8 = Cs.astype(ml_dtypes.float8_e4m3)
    bd = np.zeros((128, 128), ml_dtypes.float8_e4m3)
    bd[:L, :L] = cs8
    bd[L:, L:] = cs8
    return bd


def pack_shard(Q, csb):
    """(256, 64, 128) f32 -> (128, 1+NB, 2, 128) fp8: block 0 row 0 is
    csb; block 1+kb holds the interleaved-reversed weight stream
    t = 2j+i -> 64*Q[pair 2kb+i][p, 127-j]."""
    arr = (Q.reshape(NPAIRS, 2, L, H) * QSCALE).astype(ml_dtypes.float8_e4m3)
    pairs = np.ascontiguousarray(arr.transpose(1, 2, 0, 3).reshape(128, NPAIRS, H))
    tmp = pairs.reshape(128, NB, 2, H)[:, :, :, ::-1]       # [p, kb, i, j]
    qw = np.zeros((128, 1 + NB, 2, H), ml_dtypes.float8_e4m3)
    qw[:, 0, 0, :] = csb
    qw[:, 1:, :, :] = tmp.transpose(0, 1, 3, 2).reshape(128, NB, 2, H)
    return qw


def make_in_maps(P, d_error, circuit_edge_pairs):
    P = np.asarray(P)
    csb = make_count_matrix(circuit_edge_pairs)
    # R's output columns come out h-reversed (q = 127 - h): flip d to match
    dmat = np.ascontiguousarray(
        np.asarray(d_error)[:, ::-1], dtype=np.float32
    )
    in_maps = []
    for c in range(N_CORES):
        shard = np.ascontiguousarray(
            P[c * BPC : (c + 1) * BPC, :L, :], dtype=np.float32
        )
        in_maps.append({"qw": pack_shard(shard, csb), "dmat": dmat})
    return in_maps


def reduce_results(per_core_res, d_error=None):
    total = 0.0
    for r in per_core_res:
        total += float(np.asarray(r).astype(np.float64).sum())
    out = total / (B * E * QSCALE * QSCALE)
    return np.array(out, dtype=np.float32)


def run_spmd(P, d_error, circuit_edge_pairs, **kwargs):
    """Run on the 8 NeuronCores; returns (per-core scalars, BassKernelResults)."""
    from concourse.bass_utils import run_bass_kernel_spmd

    nc = get_nc()
    in_maps = make_in_maps(P, d_error, circuit_edge_pairs)
    res = run_bass_kernel_spmd(nc, in_maps, core_ids=list(range(N_CORES)), **kwargs)
    per_core = [res.results[c]["res_out"] for c in range(N_CORES)]
    return per_core, res


def kernel(P, d_error, circuit_edge_pairs, num_logical):
    assert int(num_logical) == L
    per_core, _ = run_spmd(P, d_error, circuit_edge_pairs)
    return reduce_results(per_core)


# revision 26
# speedup vs baseline: 1.5639x; 1.5639x over previous
"""Trainium2 Bass kernel for nn_ErrorAwareEdgeLoss.

reference:  cost[b,e] = sum_{p,q} P[b,i_e,p] * d_error[p,q] * P[b,j_e,q]
            result    = mean_{b,e} cost[b,e]

The edge pairs only enter through the count matrix
    C[l1,l2] = #edges e with (i_e,j_e) == (l1,l2),
and since d_error is symmetric the result collapses to
    result = <d_error, sum_b Q_b^T Cs Q_b> / (B*E),
with Q_b = P[b,:64,:] and Cs = (C + C^T)/2 (exact in fp8: half-integers).

Device work per core (256 batches, data-parallel over batch), all in fp8
(e4m3, host-packed as 64*Q — the softmax values are tiny so 64*Q stays
well inside [2^-6, 240] and the final scale divides by 64^2; empirical
rel err vs f64 reference ~2e-3, tolerance 2e-2):
  - ONE host-packed data stream `qw`, laid out in DoubleRowSwInterleave
    weight order (per 2-pair block kb, byte t = 2j+i holds
    64*Q[pair 2kb+i][p, 127-j]); block 0 carries blockdiag(Cs,Cs):
      * Y matmuls stream it as the moving operand (columns come out in
        (block, 127-h, pair-half) order, which just permutes PSUM
        columns),
      * R matmuls load it as SwInterleave weights — the contiguous
        weight read paces an R matmul at ~58ns vs ~77ns for the HW
        DoubleRow interleave, with no extra HBM traffic,
      * the host flips d_error's columns to absorb the reversed h
        order of R's output columns.
  - warmup matmuls on a zeroed scratch tile start the PE before any DMA
    lands (the HAM clock gate needs ~3.4us of sustained PE activity to
    lift the PE from 1.2 to 2.4GHz); scratch is memset on GPSIMD, which
    is the first engine free after the tile-context preamble
  - group loads all go on the ONE sync HWDGE ring so they complete
    strictly in consumption order; the lead group also carries the cs
    block so one descriptor batch covers both
  - Y = Cs @ Q via ONE blockdiag(Cs,Cs) matmul per 512-wide slab;
    PSUM->SBUF fp8 casts alternate between DVE and ACT as whole slabs
  - R += Q^T Y with K=256 fp8 DoubleRowSwInterleave matmuls (two
    batch-pairs per instruction) accumulated in PSUM f32 — R-matmuls
    run one group behind the Y-matmuls so the PE never waits on the
    casts
  - final reduce ON DEVICE: s[p] = sum_q R[p,q]*d_flip[p,q] via one DVE
    scalar_tensor_tensor (PSUM x SBUF) with a bf16 accumulator, then a
    1-column bf16 matmul against ones folds partitions, and a single
    4-byte result is written out
Host: result = sum_c scalar_c / (B*E*64^2).
"""

import sys

_TRN_REPO = "/opt/trn_rl_repo"
if _TRN_REPO not in sys.path:
    sys.path.insert(0, _TRN_REPO)

import numpy as np
import ml_dtypes

B, L, H = 2048, 64, 128     # batch, logical qubits, physical dim
E = 512                     # number of circuit edges
N_CORES = 8
BPC = B // N_CORES          # 256 batches per core
NPAIRS = BPC // 2           # 128 batch-pairs per core
NB = NPAIRS // 2            # 64 two-pair blocks per core
GROUPS = [8, 8] + [16] * 6 + [8, 8]  # pairs per load: small lead-in groups
                            # start the PE sooner, small tail groups cut
                            # the trailing R run; sum must be NPAIRS
SLAB_PAIRS = 4              # pairs per Y-matmul slab (512 moving columns)
QSCALE = 64.0               # host-side scale before fp8 cast
WARMUP_MM = 22              # dummy matmuls that keep the PE busy (ramping
                            # the HAM clock gate) until the first group
                            # load lands
D_LOAD_AFTER = 7            # emission point of the d_error load on the
                            # GPSIMD software-DGE queue (idle engine, so
                            # it never delays the sync ring)

_CACHE = {}


def _build():
    import concourse.tile as tile
    from concourse import bacc, mybir

    f32 = mybir.dt.float32
    fp8 = mybir.dt.float8e4

    nc = bacc.Bacc(None)
    # Host-packed stream: qw[p, 0, 0, :] = blockdiag(Cs,Cs)[p, :] and for
    # kb >= 1, flat byte t = 2j+i of block kb is
    # 64*Q[2*(kb-1) + i -th pair of partition p's half][p%64, 127-j].
    qw = nc.dram_tensor("qw", [128, 1 + NB, 2, H], fp8, kind="ExternalInput")
    dmat = nc.dram_tensor("dmat", [H, H], f32, kind="ExternalInput")
    res_out = nc.dram_tensor("res_out", [1, 1], f32, kind="ExternalOutput")

    with tile.TileContext(nc) as tc:
        with (
            tc.tile_pool(name="singles", bufs=1) as singles,
            tc.tile_pool(name="qbfp", bufs=10) as qbf_pool,
            tc.tile_pool(name="ybfp", bufs=3) as ybf_pool,
            tc.tile_pool(name="yps", bufs=6, space="PSUM") as yps,
            tc.tile_pool(name="rps", bufs=1, space="PSUM") as rps,
            tc.tile_pool(name="wps", bufs=1, space="PSUM") as wps,
        ):
            # PE p-state warmup: matmuls on a zeroed scratch tile, no DMA
            # dependency, so the HAM ramp starts as early as possible.
            scratch = singles.tile([128, 128], fp8)
            nc.gpsimd.memset(scratch[:, :], 0)
            bf16 = mybir.dt.bfloat16
            ones = singles.tile([128, 1], bf16)
            nc.gpsimd.memset(ones[:, :], 1.0)
            wm_psum = wps.tile([128, 128], f32)
            for _ in range(WARMUP_MM):
                nc.tensor.matmul(
                    wm_psum[:, :], lhsT=scratch[:, :], rhs=scratch[:, :],
                    start=True, stop=True, skip_group_check=True,
                )

            d_sb = singles.tile([H, H], f32)
            r_psum = rps.tile([128, H], f32)

            # All group loads go on the ONE sync-queue ring: SDMA engines
            # drain a ring FIFO, so groups complete in consumption order.
            # Group 0 also carries the cs block (kb=0).
            def load_group(b0, nblk, lead):
                if lead:
                    qbf = qbf_pool.tile([128, 1 + nblk, 2, H], fp8)
                    nc.sync.dma_start(
                        out=qbf[:, :, :, :], in_=qw[:, 0 : 1 + nblk, :, :]
                    )
                    return qbf
                qbf = qbf_pool.tile([128, nblk, 2, H], fp8)
                nc.sync.dma_start(
                    out=qbf[:, :, :, :], in_=qw[:, 1 + b0 : 1 + b0 + nblk, :, :]
                )
                return qbf

            _state = {"first": True, "cs2": None}

            def emit_y_slab(qbf, ybf, s, boff):
                yy = yps.tile([128, SLAB_PAIRS * H], f32)
                b = boff + s * (SLAB_PAIRS // 2)
                nc.tensor.matmul(
                    yy[:, :], lhsT=_state["cs2"],
                    rhs=qbf[:, b : b + SLAB_PAIRS // 2, :, :],
                    start=True, stop=True, skip_group_check=True,
                )
                # PSUM -> SBUF fp8 cast: whole slab on ONE engine,
                # alternating DVE/ACT per slab; a plain linear copy, so
                # ybf inherits the interleaved column order.
                eng = nc.vector.tensor_copy if s % 2 == 0 else nc.scalar.copy
                sb = s * (SLAB_PAIRS // 2)
                eng(ybf[:, sb : sb + SLAB_PAIRS // 2, :, :], yy[:, :])

            def emit_r_block(qwbuf, ybf, kb, boff, last):
                from concourse import mybir as mb

                first = _state["first"]
                _state["first"] = False
                nc.tensor.matmul(
                    r_psum[:, :],
                    lhsT=qwbuf[:, boff + kb, :, :],
                    rhs=ybf[:, kb, :, :].transpose([0, 2, 1]),
                    start=first, stop=last, skip_group_check=True,
                    perf_mode=mb.MatmulPerfMode.DoubleRowSwInterleave,
                )

            # Software pipeline: R-matmuls run one group behind the
            # Y-matmuls so the PE never waits on the PSUM->SBUF casts.
            assert sum(GROUPS) == NPAIRS
            prev = None
            b0 = 0
            for gi, npairs in enumerate(GROUPS):
                nblk = npairs // 2
                qbf = load_group(b0, nblk, lead=(gi == 0))
                boff = 1 if gi == 0 else 0
                if gi == 0:
                    _state["cs2"] = qbf[:, 0, 0, :]
                b0 += nblk
                # ybf keeps the Y-stream byte order: [block, j, i]
                ybf = ybf_pool.tile([128, nblk, H, 2], fp8)
                for s in range(npairs // SLAB_PAIRS):
                    emit_y_slab(qbf, ybf, s, boff)
                if prev is not None:
                    qw_, py_, po_, pn_ = prev
                    for kb in range(pn_):
                        emit_r_block(qw_, py_, kb, po_, last=False)
                if gi == D_LOAD_AFTER:
                    # software DGE on the idle GPSIMD engine: keeps the
                    # 64KB d_error load entirely off the sync ring that
                    # feeds the PE
                    nc.gpsimd.dma_start(out=d_sb[:, :], in_=dmat[:, :])
                prev = (qbf, ybf, boff, nblk)
            qw_, py_, po_, pn_ = prev
            for kb in range(pn_):
                emit_r_block(qw_, py_, kb, po_, last=(kb == pn_ - 1))

            # On-device <d_flip, R>: per-partition dot on DVE (bf16
            # accumulator), then a 1-column bf16 matmul against ones
            # folds the partition axis.
            prod = singles.tile([128, H], f32)
            acc16 = singles.tile([128, 1], bf16)
            nc.vector.scalar_tensor_tensor(
                out=prod[:, :], in0=r_psum[:, :], scalar=1.0, in1=d_sb[:, :],
                op0=mybir.AluOpType.mult, op1=mybir.AluOpType.mult,
                accum_out=acc16[:, :],
            )
            nc.tensor.matmul(
                wm_psum[0:1, 0:1], lhsT=ones[:, :], rhs=acc16[:, :],
                start=True, stop=True, skip_group_check=True,
            )
            res_sb = singles.tile([1, 1], f32)
            nc.vector.tensor_copy(res_sb[:, :], wm_psum[0:1, 0:1])
            nc.sync.dma_start(out=res_out[:, :], in_=res_sb[:, :])

    nc.compile()
    return nc


def get_nc():
    key = ("nc", "fp8v6")
    if key not in _CACHE:
        _CACHE[key] = _build()
    return _CACHE[key]


def make_count_matrix(circuit_edge_pairs):
    pairs = np.asarray(circuit_edge_pairs).astype(np.int64)
    C = np.zeros((L, L), np.float64)
    np.add.at(C, (pairs[:, 0], pairs[:, 1]), 1.0)
    Cs = (C + C.T) * 0.5
    cs8 = Cs.astype(ml_dtypes.float8_e4m3)
    bd = np.zeros((128, 128), ml_dtypes.float8_e4m3)
    bd[:L, :L] = cs8
    bd[L:, L:] = cs8
    return bd


def pack_shard(Q, csb):
    """(256, 64, 128) f32 -> (128, 1+NB, 2, 128) fp8: block 0 row 0 is
    csb; block 1+kb holds the interleaved-reversed weight stream
    t = 2j+i -> 64*Q[pair 2kb+i][p, 127-j]."""
    arr = (Q.reshape(NPAIRS, 2, L, H) * QSCALE).astype(ml_dtypes.float8_e4m3)
    pairs = np.ascontiguousarray(arr.transpose(1, 2, 0, 3).reshape(128, NPAIRS, H))
    tmp = pairs.reshape(128, NB, 2, H)[:, :, :, ::-1]       # [p, kb, i, j]
    qw = np.zeros((128, 1 + NB, 2, H), ml_dtypes.float8_e4m3)
    qw[:, 0, 0, :] = csb
    qw[:, 1:, :, :] = tmp.transpose(0, 1, 3, 2).reshape(128, NB, 2, H)
    return qw


def make_in_maps(P, d_error, circuit_edge_pairs):
    P = np.asarray(P)
    csb = make_count_matrix(circuit_edge_pairs)
    # R's output columns come out h-reversed (q = 127 - h): flip d to match
    dmat = np.ascontiguousarray(
        np.asarray(d_error)[:, ::-1], dtype=np.float32
    )
    in_maps = []
    for c in range(N_CORES):
        shard = np.ascontiguousarray(
            P[c * BPC : (c + 1) * BPC, :L, :], dtype=np.float32
        )
        in_maps.append({"qw": pack_shard(shard, csb), "dmat": dmat})
    return in_maps


def reduce_results(per_core_res, d_error=None):
    total = 0.0
    for r in per_core_res:
        total += float(np.asarray(r).reshape(()))
    out = total / (B * E * QSCALE * QSCALE)
    return np.array(out, dtype=np.float32)


def run_spmd(P, d_error, circuit_edge_pairs, **kwargs):
    """Run on the 8 NeuronCores; returns (per-core scalars, BassKernelResults)."""
    from concourse.bass_utils import run_bass_kernel_spmd

    nc = get_nc()
    in_maps = make_in_maps(P, d_error, circuit_edge_pairs)
    res = run_bass_kernel_spmd(nc, in_maps, core_ids=list(range(N_CORES)), **kwargs)
    per_core = [res.results[c]["res_out"] for c in range(N_CORES)]
    return per_core, res


def kernel(P, d_error, circuit_edge_pairs, num_logical):
    assert int(num_logical) == L
    per_core, _ = run_spmd(P, d_error, circuit_edge_pairs)
    return reduce_results(per_core)
